# revision 1
# baseline (speedup 1.0000x reference)
"""ConvTranspose2d (kernel=stride=2) as GEMM + pixel-shuffle on 8 TRN2 cores.

Problem: x (8, 512, 64, 64) f32, weight (512, 256, 2, 2), bias (256,)
         -> out (8, 256, 128, 128) f32.

Sharding: data-parallel over batch N: core b handles batch b. Weight/bias
replicated. No collectives.

Per-core GEMM: out[(i,j,o), (h,w)] = sum_c wfold[c, (i,j,o)] * x[c, (h,w)]
  K = 512 (4 k-tiles of 128 partitions)
  M = 1024 = 4 (i,j) x 256 o  (8 M-tiles: 2 o-halves x 4 (i,j))
  N = 4096 = 64x64 pixels     (8 chunks of 512 = 8 h-rows)

The pixel shuffle out[o, 2h+i, 2w+j] is folded into the PSUM->SBUF bias-add
copy (DVE/ACT write strided APs into a staging tile), so the DRAM store is
fully contiguous (8 KB runs per partition, 1 MB per DMA).

Matmuls run in fp16 (full PE rate, 1 cyc/row; plain fp32 is 4x slower) with
fp32 PSUM accumulation; the output is staged fp16 in SBUF/DRAM and upcast to
fp32 on the host, halving the dominant store traffic. End-to-end absmax
relative error ~5e-4.

Engine plan per core: PE streams 256 LDW+MM pairs (~57 us); input loads ride
the SP HWDGE ring while output stores ride the ACT ring (separate FIFOs avoid
head-of-line blocking); the pixel-shuffle bias-add copies split between DVE
(tensor_scalar_add) and ACT (Identity activation); measured ~68-79 us/launch,
cost-model prediction 67 us, DMA roofline ~61 us (fp32 out) / 38 us (fp16).
"""
import numpy as np
from contextlib import ExitStack

import concourse.tile as tile
from concourse import bacc, mybir
from concourse.bass_utils import run_bass_kernel_spmd

N_CORES = 8
IN_C, OUT_C, S = 512, 256, 2
H = W = 64
OH, OW = H * S, W * S          # 128, 128
KT = IN_C // 128               # 4 k-tiles
N_FULL = H * W                 # 4096 pixels
NCH = 512                      # N-chunk (one PSUM bank)
N_CHUNKS = N_FULL // NCH       # 8
H_PER = NCH // W               # 8 input rows per chunk
YROWS = H_PER * S              # 16 output rows per chunk
M_FOLD = OUT_C * S * S         # 1024

_built = None


def _build(repeats: int = 1, mm=True, dma_in=True, dma_out=True,
           scatter=True, in_eng="sp", out_eng="act", split_store=None,
           out_f16=True):
    # fp16 staging halves the dominant output-DMA traffic; the fp16 rounding
    # of the final values adds <= 2^-11 relative error. With fp16 rows the
    # parity-split store would produce 256 B runs (below DMA line-rate), so
    # split only the fp32 variant.
    if split_store is None:
        split_store = not out_f16
    out_dt = mybir.dt.float16 if out_f16 else mybir.dt.float32
    nc = bacc.Bacc("TRN2", debug=False, num_devices=N_CORES)
    xd = nc.dram_tensor("x", [IN_C, N_FULL], mybir.dt.float16,
                        kind="ExternalInput")
    wd = nc.dram_tensor("w", [IN_C, M_FOLD], mybir.dt.float16,
                        kind="ExternalInput")
    bd = nc.dram_tensor("b", [2, 128, 1], mybir.dt.float32,
                        kind="ExternalInput")
    od = nc.dram_tensor("out", [OUT_C, OH, OW], out_dt,
                        kind="ExternalOutput")

    engs = {"act": nc.scalar, "sp": nc.sync, "gpsimd": nc.gpsimd}
    with tile.TileContext(nc) as tc, ExitStack() as ctx:
        wpool = ctx.enter_context(tc.tile_pool(name="wp", bufs=1))
        bpool = ctx.enter_context(tc.tile_pool(name="bp", bufs=1))
        xpool = ctx.enter_context(tc.tile_pool(name="xp", bufs=3))
        spool = ctx.enter_context(tc.tile_pool(name="sp", bufs=2))
        ppool = ctx.enter_context(tc.tile_pool(name="pp", bufs=8, space="PSUM"))

        xda = xd.ap().rearrange("(t p) n -> t p n", p=128)
        wda = wd.ap().rearrange("(t p) m -> t p m", p=128)

        # Weights first: the first matmul group needs w0..w3; bias isn't
        # read until the first scatter (~10 us in).
        wts = []
        for k in range(KT):
            t = wpool.tile([128, M_FOLD], mybir.dt.float16, tag=f"w{k}")
            nc.sync.dma_start(t[:], wda[k])
            wts.append(t)

        bts = []
        for g in range(2):
            t = bpool.tile([128, 1], mybir.dt.float32, tag=f"bias{g}")
            nc.sync.dma_start(t[:], bd.ap()[g])
            bts.append(t)

        def body():
            for nc2 in range(N_CHUNKS // 2):
                xts2 = []
                for k in range(KT):
                    xt = xpool.tile([128, 2 * NCH], mybir.dt.float16,
                                    tag=f"x{k}")
                    if dma_in:
                        # Separate ring from the output stores: keeps x
                        # prefetch out of a FIFO where it would queue behind
                        # 1 MB stores that wait on scatter completion.
                        engs[in_eng].dma_start(
                            xt[:],
                            xda[k][:, nc2 * 2 * NCH:(nc2 + 1) * 2 * NCH])
                    else:
                        nc.gpsimd.memset(xt[:, 0:8], 0.0)
                    xts2.append(xt)
                for sub in range(2):
                    nci = nc2 * 2 + sub
                    xts = [xt[:, sub * NCH:(sub + 1) * NCH] for xt in xts2]
                    _chunk(nci, xts)

        def _chunk(nci, xts):
                for g in range(2):
                    st = spool.tile([128, YROWS * OW], out_dt,
                                    tag=f"s{g}")
                    if not scatter and dma_out:
                        nc.gpsimd.memset(st[:, 0:8], 0.0)
                    s5 = st[:].rearrange("p (h i w j) -> p h i w j",
                                         i=S, w=W, j=S)
                    for ij in range(4):
                        i, j = ij // 2, ij % 2
                        m0 = ij * OUT_C + g * 128
                        if not (mm or scatter):
                            continue
                        pt = ppool.tile([128, NCH], mybir.dt.float32,
                                        tag="ps")
                        if mm:
                            for k in range(KT):
                                nc.tensor.matmul(pt[:],
                                                 wts[k][:, m0:m0 + 128],
                                                 xts[k][:],
                                                 start=(k == 0),
                                                 stop=(k == KT - 1))
                        src = pt[:].rearrange("p (h w) -> p h w", w=W)
                        dst = s5[:, :, i, :, j]
                        if scatter:
                            if ij % 2 == 0:
                                nc.vector.tensor_scalar_add(dst, src,
                                                            bts[g][:, 0:1])
                            else:
                                nc.scalar.add(dst, src, bts[g][:, 0:1])
                    if dma_out:
                        od3 = od.ap()[g * 128:(g + 1) * 128,
                                      nci * YROWS:(nci + 1) * YROWS, :]
                        if split_store:
                            # Split the store by output-row parity: even rows
                            # (i=0) are complete after the first two scatters,
                            # so that half-store overlaps the i=1 scatters and
                            # the kernel tail shrinks to one 512 KB store.
                            st3 = st[:].rearrange("p (h two x) -> p h two x",
                                                  two=2, x=OW)
                            od4 = od3.rearrange("p (h two) x -> p h two x",
                                                two=2)
                            for par in range(2):
                                engs[out_eng].dma_start(od4[:, :, par, :],
                                                        st3[:, :, par, :])
                        else:
                            engs[out_eng].dma_start(
                                od3, st[:].rearrange("p (y x) -> p y x",
                                                     x=OW))

        if repeats == 1:
            body()
        else:
            with tc.For_i(0, repeats, 1):
                body()

    nc.compile()
    return nc


def kernel(x: np.ndarray, weight: np.ndarray, bias: np.ndarray) -> np.ndarray:
    global _built
    if _built is None:
        _built = _build()
    nc = _built

    x = np.ascontiguousarray(np.asarray(x, dtype=np.float32))
    weight = np.asarray(weight, dtype=np.float32)
    bias = np.asarray(bias, dtype=np.float32)

    # [c, o, i, j] -> [c, (i j o)] so an M-tile of 128 is one o-half of one
    # (i, j) tap: partition dim of the GEMM output is o (bias per partition,
    # contiguous DRAM rows per o). fp16 inputs: PE runs fp16 at full rate
    # (1 cyc/row vs fp32's 4) and input DMA bytes halve; absmax rel err for
    # the K=512 dot products is ~3e-4 (fp32 accumulate in PSUM).
    wfold = np.ascontiguousarray(
        weight.transpose(0, 2, 3, 1).reshape(IN_C, M_FOLD).astype(np.float16))
    bfold = np.ascontiguousarray(bias.reshape(2, 128, 1))

    in_maps = [
        {"x": np.ascontiguousarray(x[b].reshape(IN_C, N_FULL).astype(np.float16)),
         "w": wfold, "b": bfold}
        for b in range(N_CORES)
    ]
    res = run_bass_kernel_spmd(nc, in_maps, core_ids=list(range(N_CORES)))
    out = np.stack([res.results[b]["out"] for b in range(N_CORES)], axis=0)
    return np.ascontiguousarray(out.astype(np.float32))

